# revision 30
# baseline (speedup 1.0000x reference)
"""AFT-attention (nn_AFTAttention) distributed Bass kernel for 8 TRN2 NeuronCores.

Reference computation (B=8, T=4096, D=H=1024):
    Q = x @ Wq.T + bq ; K = x @ Wk.T + bk ; V = x @ Wv.T + bv      # [B,T,H]
    numer = exp(K + wbias)                                          # [B,T,H]
    denom = numer.sum(axis=0)                                       # [T,H]
    weighted = (numer * V).sum(axis=0) / denom                      # [T,H]
    out = sigmoid(Q) * weighted                                     # [B,T,H]

Sharding: the reductions are over the BATCH axis only, so sharding T across
the 8 cores makes every reduction core-local -- zero collectives.  Each core
handles all 8 batches for its 512-timestep slice.

Per-core layout: tiles are [h(128 partitions), t(512 free)] so every
per-feature constant (bq, bk+wbias, bv) rides the per-partition scalar port
of ScalarE activation / DVE scalar_tensor_tensor.  Host pre-transposes x to
x^T[b, d, t] and weights to W^T[d, h], and converts to bf16 (matmul runs at
1 col/cycle bf16 vs 4 cycles fp32).  sigmoid is computed via tanh (same ACT
table set as exp -> no 2.7us table reloads):
    sigmoid(q) * w == (tanh(q/2) + 1) * (w/2)
"""

import sys

import numpy as np

for _p in ("/opt/trn_rl_repo", "/opt/pypackages"):
    if _p not in sys.path:
        sys.path.append(_p)

B, T, D, H = 8, 4096, 1024, 1024
NCORES = 8
TC = T // NCORES      # 512 timesteps per core
P = 128               # partitions
NCH = D // P          # 8 contraction chunks
NHT = H // P          # 8 h tiles
BG = 4                # batch group size for PSUM bank pressure (Q phase)
BGA = 2               # batch group size for the K/V phases

_cached = None        # (nc, run_fn)


def _build_bass():
    import concourse.bass as bass
    import concourse.mybir as mybir
    import concourse.tile as tile
    from concourse import bacc

    f32 = mybir.dt.float32
    bf16 = mybir.dt.bfloat16
    AF = mybir.ActivationFunctionType
    OP = mybir.AluOpType

    nc = bacc.Bacc(None)

    xt = nc.declare_dram_parameter("xt", [B, NCH, P, TC], bf16, isOutput=False)
    wk = nc.declare_dram_parameter("wk", [NCH, P, H], bf16, isOutput=False)
    wv = nc.declare_dram_parameter("wv", [NCH, P, H], bf16, isOutput=False)
    wq = nc.declare_dram_parameter("wq", [NCH, P, H], bf16, isOutput=False)
    # per-partition constants, host-prearranged as [P, NHT]
    bkw = nc.declare_dram_parameter("bkw", [P, NHT], f32, isOutput=False)  # bk + wbias
    bvp = nc.declare_dram_parameter("bvp", [P, NHT], f32, isOutput=False)  # bv
    bqh = nc.declare_dram_parameter("bqh", [P, NHT], f32, isOutput=False)  # 0.5*bq
    outt = nc.declare_dram_parameter("outt", [B, H, TC], bf16, isOutput=True)

    from contextlib import ExitStack

    with tile.TileContext(nc) as tc, ExitStack() as ctx:
        sing = ctx.enter_context(tc.tile_pool(name="sing", bufs=1))
        ps = ctx.enter_context(tc.tile_pool(name="ps", bufs=8, space="PSUM"))
        acc = ctx.enter_context(tc.tile_pool(name="acc", bufs=2))
        tmp = ctx.enter_context(tc.tile_pool(name="tmp", bufs=6))
        outp = ctx.enter_context(tc.tile_pool(name="outp", bufs=6))

        # --- resident inputs -------------------------------------------------
        # weights: [P, NCH, H] per matrix; lhsT slice = w_sb[:, c, j*P:(j+1)*P]
        wk_sb = sing.tile([P, NCH, H], bf16)
        wv_sb = sing.tile([P, NCH, H], bf16)
        wq_sb = sing.tile([P, NCH, H], bf16)
        # x^T: [P, B, NCH, TC]; rhs slice = xt_sb[:, b, c, :]
        xt_sb = sing.tile([P, B, NCH, TC], bf16)
        bkw_sb = sing.tile([P, NHT], f32)
        bvp_sb = sing.tile([P, NHT], f32)
        bqh_sb = sing.tile([P, NHT], f32)

        # PE warm-up: ~10 dummy matmuls on never-written SBUF scratch issue
        # immediately after the preamble (no data deps), so the HAM clock
        # gate reaches 8/8 before the first real matmul's data lands.
        warm_w = sing.tile([P, P], bf16)
        warm_x = sing.tile([P, TC], bf16)
        nc.vector.memset(warm_w, 0.0)
        nc.vector.memset(warm_x, 0.0)
        warm_ps = ps.tile([P, TC], f32, tag="ps", name="warm_ps")
        for _ in range(10):
            nc.tensor.matmul(warm_ps, warm_w, warm_x, start=True, stop=True)
        # also pull the ~2.7us ACT table load (exp/tanh set) into the DMA
        # shadow instead of paying it on the first real exp
        warm_act = sing.tile([P, 1], f32)
        nc.scalar.activation(out=warm_act, in_=warm_w[:, 0:1], func=AF.Exp)

        # batched per-chunk DMAs (one per (c, batch-group): ~512KB each),
        # emitted in first-consumer order so the K matmuls of (j=0, g=0)
        # can start as soon as wk[c]+xt[b0..3][c] land; biases are not needed
        # until the first exp (after the first K chunk loop), so they load
        # after the first chunk pair
        for c in range(NCH):
            nc.sync.dma_start(out=wk_sb[:, c, :], in_=wk[c])
            nc.sync.dma_start(
                out=xt_sb[:, 0:BGA, c, :],
                in_=xt[0:BGA, c].rearrange("b p t -> p b t"),
            )
            nc.sync.dma_start(out=wv_sb[:, c, :], in_=wv[c])
            if c == 0:
                nc.sync.dma_start(out=bkw_sb, in_=bkw[:, :])
                nc.sync.dma_start(out=bvp_sb, in_=bvp[:, :])
                nc.sync.dma_start(out=bqh_sb, in_=bqh[:, :])
        for g in range(1, B // BGA):
            for c in range(NCH):
                nc.sync.dma_start(
                    out=xt_sb[:, g * BGA : (g + 1) * BGA, c, :],
                    in_=xt[g * BGA : (g + 1) * BGA, c].rearrange("b p t -> p b t"),
                )
        for c in range(NCH):
            nc.sync.dma_start(out=wq_sb[:, c, :], in_=wq[c])

        # --- main loops ------------------------------------------------------
        # Phase A: K/V for batch group 0 across all h tiles (only needs the
        # first half of xt -> overlaps the input DMA stream), phase B: batch
        # group 1 (completes the batch sums), phase C: Q + output.
        ns_sb = [acc.tile([P, TC], f32, tag="ns", bufs=NHT, name=f"ns_{j}") for j in range(NHT)]
        nv_sb = [acc.tile([P, TC], f32, tag="nv", bufs=NHT, name=f"nv_{j}") for j in range(NHT)]

        def kv_mms(j, bs, w_sb, out_tiles):
            hs = bass.ts(j, P)
            for c in range(NCH):
                for b in bs:
                    nc.tensor.matmul(
                        out_tiles[b], w_sb[:, c, hs], xt_sb[:, b, c, :],
                        start=(c == 0), stop=(c == NCH - 1),
                    )

        def kv_epilogue(j, bs, kps, vps):
            # numer = exp(k + bk + wbias); ns += numer; nv += numer*(v + bv)
            for b in bs:
                if b == 0:
                    numer = ns_sb[j]  # first batch: exp writes the accumulator
                else:
                    numer = tmp.tile([P, TC], f32, tag="numer", bufs=3,
                                     name=f"num_{j}_{b}")
                nc.scalar.activation(
                    out=numer, in_=kps[b], func=AF.Exp,
                    bias=bkw_sb[:, j : j + 1], scale=1.0,
                )
                if b == 0:
                    nvt = nv_sb[j]
                else:
                    nvt = tmp.tile([P, TC], f32, tag="nvt", bufs=3,
                                   name=f"nvt_{j}_{b}")
                nc.vector.scalar_tensor_tensor(
                    out=nvt, in0=vps[b], scalar=bvp_sb[:, j : j + 1],
                    in1=numer, op0=OP.add, op1=OP.mult,
                )
                if b != 0:
                    nc.gpsimd.tensor_add(ns_sb[j], ns_sb[j], numer)
                    nc.vector.tensor_add(nv_sb[j], nv_sb[j], nvt)

        for g in range(B // BGA):
            bs = range(g * BGA, (g + 1) * BGA)
            if g == 0:
                # K-lead-2 pipeline: the wv DMA stream trails wk+xt, so run
                # K(j), K(j+1), K(j+2) ahead of V(j).  PSUM peak: 3 kps
                # pairs + 1 vps pair = exactly 8 banks.
                kq = {}
                for j in (0, 1):
                    kq[j] = {b: ps.tile([P, TC], f32, tag="ps", name=f"kps_{j}_{b}") for b in bs}
                    kv_mms(j, bs, wk_sb, kq[j])
                for j in range(NHT):
                    if j + 2 < NHT:
                        kq[j + 2] = {b: ps.tile([P, TC], f32, tag="ps", name=f"kps_{j+2}_{b}") for b in bs}
                        kv_mms(j + 2, bs, wk_sb, kq[j + 2])
                    vps = {b: ps.tile([P, TC], f32, tag="ps", name=f"vps_{j}_{b}") for b in bs}
                    kv_mms(j, bs, wv_sb, vps)
                    kv_epilogue(j, bs, kq.pop(j), vps)
            else:
                for j in range(NHT):
                    kps = {b: ps.tile([P, TC], f32, tag="ps", name=f"kps_{j}_{b}") for b in bs}
                    vps = {b: ps.tile([P, TC], f32, tag="ps", name=f"vps_{j}_{b}") for b in bs}
                    kv_mms(j, bs, wk_sb, kps)
                    kv_mms(j, bs, wv_sb, vps)
                    kv_epilogue(j, bs, kps, vps)

        # weighted_half(j) = 0.5 * nv / ns, then phase C: Q + out
        wh = [tmp.tile([P, TC], bf16, tag="wh", bufs=NHT, name=f"wh_{j}") for j in range(NHT)]
        for j in range(NHT):
            rec = tmp.tile([P, TC], f32, tag="rec", bufs=2)
            nc.vector.reciprocal(rec, ns_sb[j])
            nc.vector.scalar_tensor_tensor(
                out=wh[j], in0=nv_sb[j], scalar=0.5, in1=rec, op0=OP.mult, op1=OP.mult,
            )

        for j in range(NHT):
            hs = bass.ts(j, P)
            # Q matmuls + out = (tanh(q/2 + bq/2) + 1) * weighted_half
            # (last h tile: smaller trailing groups shorten the kernel tail)
            groups = [range(0, 4), range(4, 8)] if j < NHT - 1 else [
                range(0, 4), range(4, 6), range(6, 8)]
            for bs in groups:
                qps = {b: ps.tile([P, TC], f32, tag="ps", name=f"qps_{j}_{b}") for b in bs}
                for c in range(NCH):
                    for b in bs:
                        nc.tensor.matmul(
                            qps[b], wq_sb[:, c, hs], xt_sb[:, b, c, :],
                            start=(c == 0), stop=(c == NCH - 1),
                        )
                for b in bs:
                    th = tmp.tile([P, TC], f32, tag="th", bufs=4)
                    nc.scalar.activation(
                        out=th, in_=qps[b], func=AF.Tanh,
                        bias=bqh_sb[:, j : j + 1], scale=0.5,
                    )
                    ot = outp.tile([P, TC], bf16, tag="ot")
                    nc.vector.scalar_tensor_tensor(
                        out=ot, in0=th, scalar=1.0, in1=wh[j], op0=OP.add, op1=OP.mult,
                    )
                    nc.sync.dma_start(out=outt[b, hs, :], in_=ot)

    nc.finalize()
    _dedup_ldweights(nc)
    return nc


def _dedup_ldweights(nc):
    """Drop InstLdweights that reload the exact weights already resident in
    the PE array (walrus's ldw-opt is disabled in this container, so every
    matmul otherwise gets its own LDWEIGHTS).  Sync carried by a removed
    LDWEIGHTS is preserved on an InstEventSemaphore in its place."""
    import concourse.mybir as mybir

    for bb in nc.m.functions[0].blocks:
        insts = list(bb.instructions)
        new = []
        prev_key = None
        changed = False
        for inst in insts:
            tname = type(inst).__name__
            if str(inst.engine) != "EngineType.PE":
                new.append(inst)
                continue
            if tname == "InstLdweights":
                key = (
                    str(inst.ins[0]),
                    str(inst.perf_mode),
                    str(inst.is_transpose),
                    str(inst.tile_position),
                )
                if key == prev_key:
                    si = inst.sync_info
                    if si is not None and (si.on_wait or si.on_update):
                        new.append(
                            mybir.InstEventSemaphore(
                                name=inst.name,
                                engine=inst.engine,
                                sync_info=si,
                                ins=[],
                                outs=[],
                            )
                        )
                    changed = True
                    continue
                prev_key = key
                new.append(inst)
            elif tname == "InstMatmult":
                new.append(inst)
            else:
                prev_key = None  # branches/drains: be conservative
                new.append(inst)
        if changed:
            del bb.instructions[:]
            for inst in new:
                bb.add_instruction(inst)


def _prepare_in_maps(x, Wq, bq, Wk, bk, Wv, bv, wbias):
    import ml_dtypes

    bf16 = ml_dtypes.bfloat16
    f32 = np.float32

    # weights: W.T [D, H] -> [NCH, P, H] bf16 (shared by all cores)
    def prep_w(w):
        return np.ascontiguousarray(w.T.astype(bf16)).reshape(NCH, P, H)

    wq_h = prep_w(np.asarray(Wq))
    wk_h = prep_w(np.asarray(Wk))
    wv_h = prep_w(np.asarray(Wv))

    # per-partition constants as [P, NHT]: col j holds values for h in [j*128, ...)
    def prep_b(v):
        return np.ascontiguousarray(np.asarray(v, f32).reshape(NHT, P).T)

    bkw_h = prep_b(np.asarray(bk, f32) + np.asarray(wbias, f32))
    bvp_h = prep_b(bv)
    bqh_h = prep_b(0.5 * np.asarray(bq, f32))

    x = np.asarray(x)
    in_maps = []
    for core in range(NCORES):
        xs = x[:, core * TC : (core + 1) * TC, :]           # [B, TC, D]
        xtc = np.ascontiguousarray(xs.transpose(0, 2, 1).astype(bf16)).reshape(
            B, NCH, P, TC
        )
        in_maps.append(
            {
                "xt": xtc,
                "wq": wq_h,
                "wk": wk_h,
                "wv": wv_h,
                "bkw": bkw_h,
                "bvp": bvp_h,
                "bqh": bqh_h,
            }
        )
    return in_maps


def _get_nc():
    global _cached
    if _cached is None:
        _cached = _build_bass()
    return _cached


TRACE = False          # set True from a test harness to profile
TRACE_TMPDIR = None    # optional persistent dir for trace artifacts
LAST_RESULT = None     # BassKernelResults of the most recent kernel() call


def kernel(x, Wq, bq, Wk, bk, Wv, bv, wbias):
    global LAST_RESULT
    from concourse.bass_utils import run_bass_kernel_spmd

    nc = _get_nc()
    in_maps = _prepare_in_maps(x, Wq, bq, Wk, bk, Wv, bv, wbias)
    kw = {}
    if TRACE:
        kw = {"trace": True, "tmpdir": TRACE_TMPDIR}
    res = run_bass_kernel_spmd(nc, in_maps, core_ids=list(range(NCORES)), **kw)
    LAST_RESULT = res
    out = np.empty((B, T, H), np.float32)
    for core in range(NCORES):
        o = np.asarray(res.results[core]["outt"])            # [B, H, TC] bf16
        out[:, core * TC : (core + 1) * TC, :] = o.astype(np.float32).transpose(
            0, 2, 1
        )
    return out


# revision 32
# speedup vs baseline: 1.0156x; 1.0156x over previous
"""AFT-attention (nn_AFTAttention) distributed Bass kernel for 8 TRN2 NeuronCores.

Reference computation (B=8, T=4096, D=H=1024):
    Q = x @ Wq.T + bq ; K = x @ Wk.T + bk ; V = x @ Wv.T + bv      # [B,T,H]
    numer = exp(K + wbias)                                          # [B,T,H]
    denom = numer.sum(axis=0)                                       # [T,H]
    weighted = (numer * V).sum(axis=0) / denom                      # [T,H]
    out = sigmoid(Q) * weighted                                     # [B,T,H]

Sharding: the reductions are over the BATCH axis only, so sharding T across
the 8 cores makes every reduction core-local -- zero collectives.  Each core
handles all 8 batches for its 512-timestep slice.

Per-core layout: tiles are [h(128 partitions), t(512 free)] so every
per-feature constant (bq, bk+wbias, bv) rides the per-partition scalar port
of ScalarE activation / DVE scalar_tensor_tensor.  Host pre-transposes x to
x^T[b, d, t] and weights to W^T[d, h], and converts to bf16 (matmul runs at
1 col/cycle bf16 vs 4 cycles fp32).  sigmoid is computed via tanh (same ACT
table set as exp -> no 2.7us table reloads):
    sigmoid(q) * w == (tanh(q/2) + 1) * (w/2)
"""

import sys

import numpy as np

for _p in ("/opt/trn_rl_repo", "/opt/pypackages"):
    if _p not in sys.path:
        sys.path.append(_p)

B, T, D, H = 8, 4096, 1024, 1024
NCORES = 8
TC = T // NCORES      # 512 timesteps per core
P = 128               # partitions
NCH = D // P          # 8 contraction chunks
NHT = H // P          # 8 h tiles
BG = 4                # batch group size for PSUM bank pressure (Q phase)
BGA = 2               # batch group size for the K/V phases

_cached = None        # (nc, run_fn)


def _build_bass():
    import concourse.bass as bass
    import concourse.mybir as mybir
    import concourse.tile as tile
    from concourse import bacc

    f32 = mybir.dt.float32
    bf16 = mybir.dt.bfloat16
    AF = mybir.ActivationFunctionType
    OP = mybir.AluOpType

    nc = bacc.Bacc(None)

    xt = nc.declare_dram_parameter("xt", [B, NCH, P, TC], bf16, isOutput=False)
    wk = nc.declare_dram_parameter("wk", [NCH, P, H], bf16, isOutput=False)
    wv = nc.declare_dram_parameter("wv", [NCH, P, H], bf16, isOutput=False)
    wq = nc.declare_dram_parameter("wq", [NCH, P, H], bf16, isOutput=False)
    # per-partition constants, host-prearranged as [P, NHT]
    bkw = nc.declare_dram_parameter("bkw", [P, NHT], f32, isOutput=False)  # bk + wbias
    bvp = nc.declare_dram_parameter("bvp", [P, NHT], f32, isOutput=False)  # bv
    bqh = nc.declare_dram_parameter("bqh", [P, NHT], f32, isOutput=False)  # 0.5*bq
    outt = nc.declare_dram_parameter("outt", [B, H, TC], bf16, isOutput=True)

    from contextlib import ExitStack

    with tile.TileContext(nc) as tc, ExitStack() as ctx:
        sing = ctx.enter_context(tc.tile_pool(name="sing", bufs=1))
        ps = ctx.enter_context(tc.tile_pool(name="ps", bufs=8, space="PSUM"))
        acc = ctx.enter_context(tc.tile_pool(name="acc", bufs=2))
        tmp = ctx.enter_context(tc.tile_pool(name="tmp", bufs=6))
        outp = ctx.enter_context(tc.tile_pool(name="outp", bufs=6))

        # --- resident inputs -------------------------------------------------
        # weights: [P, NCH, H] per matrix; lhsT slice = w_sb[:, c, j*P:(j+1)*P]
        wk_sb = sing.tile([P, NCH, H], bf16)
        wv_sb = sing.tile([P, NCH, H], bf16)
        wq_sb = sing.tile([P, NCH, H], bf16)
        # x^T: [P, B, NCH, TC]; rhs slice = xt_sb[:, b, c, :]
        xt_sb = sing.tile([P, B, NCH, TC], bf16)
        bkw_sb = sing.tile([P, NHT], f32)
        bvp_sb = sing.tile([P, NHT], f32)
        bqh_sb = sing.tile([P, NHT], f32)

        # PE warm-up: ~10 dummy matmuls on never-written SBUF scratch issue
        # immediately after the preamble (no data deps), so the HAM clock
        # gate reaches 8/8 before the first real matmul's data lands.
        warm_w = sing.tile([P, P], bf16)
        warm_x = sing.tile([P, TC], bf16)
        nc.vector.memset(warm_w, 0.0)
        nc.vector.memset(warm_x, 0.0)
        warm_ps = ps.tile([P, TC], f32, tag="ps", name="warm_ps")
        for _ in range(10):
            nc.tensor.matmul(warm_ps, warm_w, warm_x, start=True, stop=True)
        # also pull the ~2.7us ACT table load (exp/tanh set) into the DMA
        # shadow instead of paying it on the first real exp
        warm_act = sing.tile([P, 1], f32)
        nc.scalar.activation(out=warm_act, in_=warm_w[:, 0:1], func=AF.Exp)

        # batched per-chunk DMAs (one per (c, batch-group): ~512KB each),
        # emitted in first-consumer order so the K matmuls of (j=0, g=0)
        # can start as soon as wk[c]+xt[b0..3][c] land; biases are not needed
        # until the first exp (after the first K chunk loop), so they load
        # after the first chunk pair
        for c in range(NCH):
            nc.sync.dma_start(out=wk_sb[:, c, :], in_=wk[c])
            nc.sync.dma_start(
                out=xt_sb[:, 0:BGA, c, :],
                in_=xt[0:BGA, c].rearrange("b p t -> p b t"),
            )
            if c == 0:
                nc.sync.dma_start(out=bkw_sb, in_=bkw[:, :])
                nc.sync.dma_start(out=bvp_sb, in_=bvp[:, :])
                nc.sync.dma_start(out=bqh_sb, in_=bqh[:, :])
        for c in range(NCH):
            nc.sync.dma_start(out=wv_sb[:, c, :], in_=wv[c])
        for g in range(1, B // BGA):
            for c in range(NCH):
                nc.sync.dma_start(
                    out=xt_sb[:, g * BGA : (g + 1) * BGA, c, :],
                    in_=xt[g * BGA : (g + 1) * BGA, c].rearrange("b p t -> p b t"),
                )
        for c in range(NCH):
            nc.sync.dma_start(out=wq_sb[:, c, :], in_=wq[c])

        # --- main loops ------------------------------------------------------
        # Phase A: K/V for batch group 0 across all h tiles (only needs the
        # first half of xt -> overlaps the input DMA stream), phase B: batch
        # group 1 (completes the batch sums), phase C: Q + output.
        ns_sb = [acc.tile([P, TC], f32, tag="ns", bufs=NHT, name=f"ns_{j}") for j in range(NHT)]
        nv_sb = [acc.tile([P, TC], f32, tag="nv", bufs=NHT, name=f"nv_{j}") for j in range(NHT)]

        def kv_mms(j, bs, w_sb, out_tiles):
            hs = bass.ts(j, P)
            for c in range(NCH):
                for b in bs:
                    nc.tensor.matmul(
                        out_tiles[b], w_sb[:, c, hs], xt_sb[:, b, c, :],
                        start=(c == 0), stop=(c == NCH - 1),
                    )

        def kv_epilogue(j, bs, kps, vps):
            # numer = exp(k + bk + wbias); ns += numer; nv += numer*(v + bv)
            for b in bs:
                if b == 0:
                    numer = ns_sb[j]  # first batch: exp writes the accumulator
                else:
                    numer = tmp.tile([P, TC], f32, tag="numer", bufs=6,
                                     name=f"num_{j}_{b}")
                nc.scalar.activation(
                    out=numer, in_=kps[b], func=AF.Exp,
                    bias=bkw_sb[:, j : j + 1], scale=1.0,
                )
                if b == 0:
                    nvt = nv_sb[j]
                else:
                    nvt = tmp.tile([P, TC], f32, tag="nvt", bufs=3,
                                   name=f"nvt_{j}_{b}")
                nc.vector.scalar_tensor_tensor(
                    out=nvt, in0=vps[b], scalar=bvp_sb[:, j : j + 1],
                    in1=numer, op0=OP.add, op1=OP.mult,
                )
                if b != 0:
                    nc.gpsimd.tensor_add(ns_sb[j], ns_sb[j], numer)
                    nc.vector.tensor_add(nv_sb[j], nv_sb[j], nvt)

        for g in range(B // BGA):
            bs = range(g * BGA, (g + 1) * BGA)
            if g == 0:
                # Chunk-outer waves: 4 h-tiles x 2 batches = 8 PSUM banks
                # accumulate together, consuming each (wk[c], xt[c]) DMA the
                # moment it lands (no head-of-line block on a later chunk).
                # K waves run first so the trailing wv stream has time.
                numer_st = {}

                def k_wave(jlist):
                    kq = {j: {b: ps.tile([P, TC], f32, tag="ps", name=f"kps_{j}_{b}") for b in bs}
                          for j in jlist}
                    for c in range(NCH):
                        for j in jlist:
                            for b in bs:
                                nc.tensor.matmul(
                                    kq[j][b], wk_sb[:, c, bass.ts(j, P)], xt_sb[:, b, c, :],
                                    start=(c == 0), stop=(c == NCH - 1),
                                )
                    for j in jlist:
                        for b in bs:
                            if b == 0:
                                numer = ns_sb[j]
                            else:
                                numer = tmp.tile([P, TC], f32, tag="numer", bufs=6,
                                                 name=f"numw_{j}_{b}")
                            nc.scalar.activation(
                                out=numer, in_=kq[j][b], func=AF.Exp,
                                bias=bkw_sb[:, j : j + 1], scale=1.0,
                            )
                            numer_st[(j, b)] = numer

                def v_wave(jlist):
                    vq = {j: {b: ps.tile([P, TC], f32, tag="ps", name=f"vps_{j}_{b}") for b in bs}
                          for j in jlist}
                    for c in range(NCH):
                        for j in jlist:
                            for b in bs:
                                nc.tensor.matmul(
                                    vq[j][b], wv_sb[:, c, bass.ts(j, P)], xt_sb[:, b, c, :],
                                    start=(c == 0), stop=(c == NCH - 1),
                                )
                    for j in jlist:
                        for b in bs:
                            numer = numer_st.pop((j, b))
                            if b == 0:
                                nvt = nv_sb[j]
                            else:
                                nvt = tmp.tile([P, TC], f32, tag="nvt", bufs=3,
                                               name=f"nvtw_{j}_{b}")
                            nc.vector.scalar_tensor_tensor(
                                out=nvt, in0=vq[j][b], scalar=bvp_sb[:, j : j + 1],
                                in1=numer, op0=OP.add, op1=OP.mult,
                            )
                            if b != 0:
                                nc.gpsimd.tensor_add(ns_sb[j], ns_sb[j], numer)
                                nc.vector.tensor_add(nv_sb[j], nv_sb[j], nvt)

                k_wave(range(0, 4))
                v_wave(range(0, 4))
                k_wave(range(4, 8))
                v_wave(range(4, 8))
            else:
                for j in range(NHT):
                    kps = {b: ps.tile([P, TC], f32, tag="ps", name=f"kps_{j}_{b}") for b in bs}
                    vps = {b: ps.tile([P, TC], f32, tag="ps", name=f"vps_{j}_{b}") for b in bs}
                    kv_mms(j, bs, wk_sb, kps)
                    kv_mms(j, bs, wv_sb, vps)
                    kv_epilogue(j, bs, kps, vps)

        # weighted_half(j) = 0.5 * nv / ns, then phase C: Q + out
        wh = [tmp.tile([P, TC], bf16, tag="wh", bufs=NHT, name=f"wh_{j}") for j in range(NHT)]
        for j in range(NHT):
            rec = tmp.tile([P, TC], f32, tag="rec", bufs=2)
            nc.vector.reciprocal(rec, ns_sb[j])
            nc.vector.scalar_tensor_tensor(
                out=wh[j], in0=nv_sb[j], scalar=0.5, in1=rec, op0=OP.mult, op1=OP.mult,
            )

        for j in range(NHT):
            hs = bass.ts(j, P)
            # Q matmuls + out = (tanh(q/2 + bq/2) + 1) * weighted_half
            # (last h tile: smaller trailing groups shorten the kernel tail)
            groups = [range(0, 4), range(4, 8)] if j < NHT - 1 else [
                range(0, 4), range(4, 6), range(6, 8)]
            for bs in groups:
                qps = {b: ps.tile([P, TC], f32, tag="ps", name=f"qps_{j}_{b}") for b in bs}
                for c in range(NCH):
                    for b in bs:
                        nc.tensor.matmul(
                            qps[b], wq_sb[:, c, hs], xt_sb[:, b, c, :],
                            start=(c == 0), stop=(c == NCH - 1),
                        )
                for b in bs:
                    th = tmp.tile([P, TC], f32, tag="th", bufs=4)
                    nc.scalar.activation(
                        out=th, in_=qps[b], func=AF.Tanh,
                        bias=bqh_sb[:, j : j + 1], scale=0.5,
                    )
                    ot = outp.tile([P, TC], bf16, tag="ot")
                    nc.vector.scalar_tensor_tensor(
                        out=ot, in0=th, scalar=1.0, in1=wh[j], op0=OP.add, op1=OP.mult,
                    )
                    nc.sync.dma_start(out=outt[b, hs, :], in_=ot)

    nc.finalize()
    _dedup_ldweights(nc)
    return nc


def _dedup_ldweights(nc):
    """Drop InstLdweights that reload the exact weights already resident in
    the PE array (walrus's ldw-opt is disabled in this container, so every
    matmul otherwise gets its own LDWEIGHTS).  Sync carried by a removed
    LDWEIGHTS is preserved on an InstEventSemaphore in its place."""
    import concourse.mybir as mybir

    for bb in nc.m.functions[0].blocks:
        insts = list(bb.instructions)
        new = []
        prev_key = None
        changed = False
        for inst in insts:
            tname = type(inst).__name__
            if str(inst.engine) != "EngineType.PE":
                new.append(inst)
                continue
            if tname == "InstLdweights":
                key = (
                    str(inst.ins[0]),
                    str(inst.perf_mode),
                    str(inst.is_transpose),
                    str(inst.tile_position),
                )
                if key == prev_key:
                    si = inst.sync_info
                    if si is not None and (si.on_wait or si.on_update):
                        new.append(
                            mybir.InstEventSemaphore(
                                name=inst.name,
                                engine=inst.engine,
                                sync_info=si,
                                ins=[],
                                outs=[],
                            )
                        )
                    changed = True
                    continue
                prev_key = key
                new.append(inst)
            elif tname == "InstMatmult":
                new.append(inst)
            else:
                prev_key = None  # branches/drains: be conservative
                new.append(inst)
        if changed:
            del bb.instructions[:]
            for inst in new:
                bb.add_instruction(inst)


def _prepare_in_maps(x, Wq, bq, Wk, bk, Wv, bv, wbias):
    import ml_dtypes

    bf16 = ml_dtypes.bfloat16
    f32 = np.float32

    # weights: W.T [D, H] -> [NCH, P, H] bf16 (shared by all cores)
    def prep_w(w):
        return np.ascontiguousarray(w.T.astype(bf16)).reshape(NCH, P, H)

    wq_h = prep_w(np.asarray(Wq))
    wk_h = prep_w(np.asarray(Wk))
    wv_h = prep_w(np.asarray(Wv))

    # per-partition constants as [P, NHT]: col j holds values for h in [j*128, ...)
    def prep_b(v):
        return np.ascontiguousarray(np.asarray(v, f32).reshape(NHT, P).T)

    bkw_h = prep_b(np.asarray(bk, f32) + np.asarray(wbias, f32))
    bvp_h = prep_b(bv)
    bqh_h = prep_b(0.5 * np.asarray(bq, f32))

    x = np.asarray(x)
    in_maps = []
    for core in range(NCORES):
        xs = x[:, core * TC : (core + 1) * TC, :]           # [B, TC, D]
        xtc = np.ascontiguousarray(xs.transpose(0, 2, 1).astype(bf16)).reshape(
            B, NCH, P, TC
        )
        in_maps.append(
            {
                "xt": xtc,
                "wq": wq_h,
                "wk": wk_h,
                "wv": wv_h,
                "bkw": bkw_h,
                "bvp": bvp_h,
                "bqh": bqh_h,
            }
        )
    return in_maps


def _get_nc():
    global _cached
    if _cached is None:
        _cached = _build_bass()
    return _cached


TRACE = False          # set True from a test harness to profile
TRACE_TMPDIR = None    # optional persistent dir for trace artifacts
LAST_RESULT = None     # BassKernelResults of the most recent kernel() call


def kernel(x, Wq, bq, Wk, bk, Wv, bv, wbias):
    global LAST_RESULT
    from concourse.bass_utils import run_bass_kernel_spmd

    nc = _get_nc()
    in_maps = _prepare_in_maps(x, Wq, bq, Wk, bk, Wv, bv, wbias)
    kw = {}
    if TRACE:
        kw = {"trace": True, "tmpdir": TRACE_TMPDIR}
    res = run_bass_kernel_spmd(nc, in_maps, core_ids=list(range(NCORES)), **kw)
    LAST_RESULT = res
    out = np.empty((B, T, H), np.float32)
    for core in range(NCORES):
        o = np.asarray(res.results[core]["outt"])            # [B, H, TC] bf16
        out[:, core * TC : (core + 1) * TC, :] = o.astype(np.float32).transpose(
            0, 2, 1
        )
    return out
